# revision 5
# baseline (speedup 1.0000x reference)
"""Bass/Trainium2 kernel for nn_BinResNetConv2d.

Computes: BatchNorm2d (inference) -> sign binarization -> 3x3 conv
(256->256 ch, stride 1, pad 1, no bias) -> ReLU on x[32, 256, 56, 56].

Strategy: data-parallel over batch across 8 NeuronCores (4 images/core,
conv weights + BN params replicated). Per core:
  - BN is folded on host into per-channel (scale, shift); one ScalarE
    activation computes sign(x*scale + shift) per tile, writing fp8e4
    +/-1 into a zero-padded [128ci, 2ci_t, 58, 64] SBUF image.
  - The 3x3 conv accumulates 15 matmuls per output tile into one PSUM
    bank: 12 bf16 matmuls (taps ky=0,1; lhsT bf16, rhs the fp8 sign
    image - mixed dtype runs at full bf16 rate) + 3 fp8 DoubleRow
    matmuls (taps ky=2; both ci halves paired per tap, virtual K=256).
    DoubleRow at N=448 measures the same 189ns/MM as bf16, so each DR
    matmul covers two K-slices for the price of one: 15 MMs vs 18.
  - All weights are pre-scaled by S=1.6 on host (the e4m3 error of the
    ky=2 taps is minimized near this scale); eviction applies
    max(psum * 1/S, 0) on VectorE, undoing S exactly in f32.

Accuracy: sign inputs are exact in fp8; the only error is bf16 rounding
(12 slices) + e4m3 rounding (6 slices) of the weights: rel-max ~1.45e-2
vs the 2e-2 gate on the reference data.
"""

import numpy as np
import ml_dtypes

N_CORES = 8
NB = 4            # images per core (32 / 8)
C = 256
H = W = 56
HP = 58           # padded rows
WROW = 64         # row stride of the padded fp8 image (16B aligned)
ROWS_PER_TILE = 8
N_ROW_TILES = H // ROWS_PER_TILE  # 7
S = 1.6           # weight pre-scale; evictions multiply by 1/S
INV_S = 1.0 / S

_nc_cache = {}
LAST_RESULTS = None


def _build_nc():
    import concourse.mybir as mybir
    import concourse.tile as tile
    from concourse import bacc

    f32 = mybir.dt.float32
    bf16 = mybir.dt.bfloat16
    f8e4 = mybir.dt.float8e4
    AF = mybir.ActivationFunctionType
    ALU = mybir.AluOpType
    DR = mybir.MatmulPerfMode.DoubleRow

    nc = bacc.Bacc("TRN2", target_bir_lowering=False, debug=False)
    x_d = nc.dram_tensor("x", (NB, C, H, W), f32, kind="ExternalInput")
    # wb[ci, co_t, ci_t, tap(ky*3+kx, ky<2), co]: bf16 lhsT slices
    wb_d = nc.dram_tensor("wb", (128, 2, 2, 6, 128), bf16,
                          kind="ExternalInput")
    # w8[ci, co_t, kx, ci_t(pair), co]: fp8 DoubleRow lhsT slices (ky=2)
    w8_d = nc.dram_tensor("w8", (128, 2, 3, 2, 128), f8e4,
                          kind="ExternalInput")
    bnp_d = nc.dram_tensor("bnp", (2, 128, 2), f32, kind="ExternalInput")
    y_d = nc.dram_tensor("y", (NB, C, H, W), f32, kind="ExternalOutput")

    with tile.TileContext(nc) as tc:
        with (
            tc.tile_pool(name="const", bufs=1) as cpool,
            tc.tile_pool(name="xp", bufs=1) as xpool,
            tc.tile_pool(name="chunk", bufs=8) as hpool,
            tc.tile_pool(name="stage", bufs=3) as spool,
            tc.tile_pool(name="out", bufs=4) as opool,
            tc.tile_pool(name="psum", bufs=8, space="PSUM") as ppool,
        ):
            # zero scratch for PE warm-up matmuls (HAM un-throttles after
            # ~3.4us of sustained PE work; run it on zeros while x loads).
            warm_sb = cpool.tile([128, 256], bf16, tag="warm")
            nc.gpsimd.memset(warm_sb[:], 0.0)
            # BN params: sole first transfer on the ScalarE HW-DGE ring so
            # nothing can starve it (the first Sign waits on it)
            bnp_sb = []  # [128, 2]: col 0 = scale, col 1 = shift
            for ci_t in range(2):
                t = cpool.tile([128, 2], f32, tag=f"bnp{ci_t}")
                nc.scalar.dma_start(t[:], bnp_d[ci_t])
                bnp_sb.append(t)
            wb_sb = cpool.tile([128, 2, 2, 6, 128], bf16, tag="wb")
            w8_sb = cpool.tile([128, 2, 3, 2, 128], f8e4, tag="w8")

            # --- padded fp8 sign images; borders zeroed (disjoint from the
            # interior Sign writes, so no dep lands on the Activation ops)
            xq = {}   # n -> [128, 2, HP, WROW] f8e4, zero border
            for n in range(NB):
                t = xpool.tile([128, 2, HP, WROW], f8e4, tag=f"xq{n}")
                nc.gpsimd.memset(t[:, :, 0, 0:H + 2], 0.0)
                nc.gpsimd.memset(t[:, :, HP - 1, 0:H + 2], 0.0)
                nc.gpsimd.memset(t[:, :, 1:HP - 1, 0], 0.0)
                nc.gpsimd.memset(t[:, :, 1:HP - 1, H + 1], 0.0)
                xq[n] = t

            def binarize(n, ci_t, r, nr, src_ap):
                """Sign(x*scale+shift) into padded rows [r, r+nr) of xq."""
                nc.scalar.activation(
                    xq[n][:, ci_t, 1 + r:1 + r + nr, 1:1 + W], src_ap,
                    AF.Sign,
                    bias=bnp_sb[ci_t][:, 1:2], scale=bnp_sb[ci_t][:, 0:1])

            # image 0 in row-chunks per ci tile: first conv matmuls can
            # start as soon as the first ~8 rows have landed + signed.
            # HBM is fair-shared across active DMA queues, so the SP ring
            # is issued in waves: each wave's first transfer must complete
            # before the next wave may issue.
            from concourse.tile import add_dep_helper

            CHUNK_ROWS = [(8, 0), (6, 8), (14, 14), (14, 28), (14, 42)]

            def chunk_dma(c, ci_t, nsub=1):
                """Load + binarize chunk c of image 0."""
                nr, r = CHUNK_ROWS[c]
                h = nr // 2 if nsub == 2 else nr
                st = hpool.tile([128, 14, W], f32, tag="chunk", name="st")
                dma = nc.sync.dma_start(
                    st[:, 0:h, :],
                    x_d[0, ci_t * 128:(ci_t + 1) * 128, r:r + h, :])
                if nsub == 2:
                    nc.sync.dma_start(
                        st[:, h:nr, :],
                        x_d[0, ci_t * 128:(ci_t + 1) * 128, r + h:r + nr, :])
                binarize(0, ci_t, r, nr, st[:, 0:nr, :])
                return dma

            def stage_dma(n, ci_t):
                st = spool.tile([128, H, W], f32, tag="stage", name="st")
                dma = nc.sync.dma_start(
                    st[:, 0:H // 2, :],
                    x_d[n, ci_t * 128:(ci_t + 1) * 128, 0:H // 2, :])
                nc.sync.dma_start(
                    st[:, H // 2:H, :],
                    x_d[n, ci_t * 128:(ci_t + 1) * 128, H // 2:H, :])
                binarize(n, ci_t, 0, H, st[:])
                return dma

            def wb_dma(co_t, ci_t, taps=(0, 6)):
                lo, hi = taps
                return nc.sync.dma_start(
                    wb_sb[:, co_t, ci_t, lo:hi], wb_d[:, co_t, ci_t, lo:hi])

            def w8_dma(co_t):
                return nc.sync.dma_start(
                    w8_sb[:, co_t], w8_d[:, co_t])

            waves = [
                # wave 1: rows 0-7 of image 0 (split over 2 queues each) +
                # only the ky=0 co0 weights -> first matmuls unblock fastest
                [lambda: chunk_dma(0, 0, nsub=2), lambda: chunk_dma(0, 1, nsub=2),
                 lambda: wb_dma(0, 0, taps=(0, 3)), lambda: wb_dma(0, 1, taps=(0, 3))],
                # image-0 chunks keep strict priority over the bulky co1
                # weights; each wave gates on the previous wave's first
                # chunk so HBM fair-sharing can't starve the early rows
                [lambda: chunk_dma(1, 0), lambda: chunk_dma(1, 1),
                 lambda: wb_dma(0, 0, taps=(3, 6)), lambda: wb_dma(0, 1, taps=(3, 6)),
                 lambda: w8_dma(0)],
                [lambda: chunk_dma(2, 0), lambda: chunk_dma(2, 1)],
                [lambda: chunk_dma(3, 0), lambda: chunk_dma(3, 1),
                 lambda: wb_dma(1, 0), lambda: wb_dma(1, 1),
                 lambda: w8_dma(1)],
                [lambda: chunk_dma(4, 0), lambda: chunk_dma(4, 1)],
                # final wave: images 1..3
                [lambda n=n, ci=ci: stage_dma(n, ci)
                 for n in range(1, NB) for ci in range(2)],
            ]
            gates = [0, 0, 0, 0, 0]
            gate = None
            for wi, wave in enumerate(waves):
                emitted = []
                for emit in wave:
                    dma = emit()
                    if gate is not None:
                        add_dep_helper(dma.ins, gate.ins, sync=True,
                                       reason="DMA wave schedule")
                    emitted.append(dma)
                if wi < len(gates):
                    gate = emitted[gates[wi]]

            # PE warm-up: zero matmuls keep the PE's activity monitor busy
            # from ~7us until the first real matmul, so conv starts at the
            # full 2.4GHz clock instead of the 1.2GHz cold state
            warm_ps = ppool.tile([128, 448], f32, tag="ps")
            last_warm = None
            for _ in range(17):
                last_warm = nc.tensor.matmul(
                    warm_ps[0:64, 0:256], warm_sb[:, 0:64], warm_sb[:])

            # --- conv: 15 accumulating matmuls per output tile ---
            # The bf16->DR PE mode transition costs ~150ns (measured); the
            # reverse ~10ns. Batch QUAD tiles: 4x12 bf16 matmuls, then the
            # 4x3 DoubleRow matmuls, then the 4 evictions -> one
            # transition per 4 tiles instead of per tile.
            n_tiles = NB * 2 * N_ROW_TILES
            tiles = [(n, co_t, rb)
                     for n in range(NB)
                     for co_t in range(2)
                     for rb in range(N_ROW_TILES)]
            QUAD = 4
            first_mm = None
            for q0 in range(0, n_tiles, QUAD):
                quad = tiles[q0:q0 + QUAD]
                pss = []
                for qi, (n, co_t, rb) in enumerate(quad):
                    r0 = rb * ROWS_PER_TILE
                    ps = ppool.tile([128, ROWS_PER_TILE, W], f32, tag="ps",
                                    name=f"ps{q0 + qi}")
                    pss.append(ps)
                    k = 0
                    for ky in range(2):
                        for ci_t in range(2):
                            for kx in range(3):
                                mm = nc.tensor.matmul(
                                    ps[:], wb_sb[:, co_t, ci_t, ky * 3 + kx],
                                    xq[n][:, ci_t,
                                          r0 + ky:r0 + ky + ROWS_PER_TILE,
                                          kx:kx + W],
                                    start=(k == 0), stop=False)
                                if first_mm is None:
                                    first_mm = mm
                                k += 1
                for qi, (n, co_t, rb) in enumerate(quad):
                    r0 = rb * ROWS_PER_TILE
                    for kx in range(3):
                        nc.tensor.matmul(
                            pss[qi][:], w8_sb[:, co_t, kx],
                            xq[n][:, 0:2,
                                  r0 + 2:r0 + 2 + ROWS_PER_TILE,
                                  kx:kx + W],
                            start=False, stop=(kx == 2), perf_mode=DR)
                for qi, (n, co_t, rb) in enumerate(quad):
                    r0 = rb * ROWS_PER_TILE
                    co_sl = slice(co_t * 128, (co_t + 1) * 128)
                    ps = pss[qi]
                    ob = opool.tile([128, ROWS_PER_TILE, W], f32, tag="ob",
                                    name=f"ob{q0 + qi}")
                    if q0 + qi >= n_tiles - 2:
                        # final tiles: evacuate + store in halves across
                        # both DMA rings so the kernel tail pipelines
                        half = ROWS_PER_TILE // 2
                        nc.vector.tensor_scalar(
                            ob[:, 0:half, :], ps[:, 0:half, :],
                            INV_S, 0.0, op0=ALU.mult, op1=ALU.max)
                        nc.sync.dma_start(
                            y_d[n, co_sl, r0:r0 + half, :],
                            ob[:, 0:half, :])
                        nc.vector.tensor_scalar(
                            ob[:, half:ROWS_PER_TILE, :],
                            ps[:, half:ROWS_PER_TILE, :],
                            INV_S, 0.0, op0=ALU.mult, op1=ALU.max)
                        nc.scalar.dma_start(
                            y_d[n, co_sl, r0 + half:r0 + ROWS_PER_TILE, :],
                            ob[:, half:ROWS_PER_TILE, :])
                    else:
                        nc.vector.tensor_scalar(
                            ob[:], ps[:], INV_S, 0.0,
                            op0=ALU.mult, op1=ALU.max)
                        nc.sync.dma_start(
                            y_d[n, co_sl, r0:r0 + ROWS_PER_TILE, :], ob[:])
            # keep warm-up strictly before the real matmuls on the PE queue
            add_dep_helper(first_mm.ins, last_warm.ins, sync=False,
                           reason="PE warm-up precedes conv")
    nc.compile()
    return nc


def _get_nc():
    if "nc" not in _nc_cache:
        _nc_cache["nc"] = _build_nc()
    return _nc_cache["nc"]


def kernel(x, w, gamma, beta, running_mean, running_var, _trace=False):
    global LAST_RESULTS
    from concourse.bass_utils import run_bass_kernel_spmd

    x = np.ascontiguousarray(np.asarray(x, dtype=np.float32))
    w = np.asarray(w, dtype=np.float32)
    gamma = np.asarray(gamma, dtype=np.float32)
    beta = np.asarray(beta, dtype=np.float32)
    running_mean = np.asarray(running_mean, dtype=np.float32)
    running_var = np.asarray(running_var, dtype=np.float32)

    # fold BN (inference) into per-channel scale/shift
    eps = 1e-5
    scale = gamma / np.sqrt(running_var + eps)
    shift = beta - running_mean * scale

    # weights -> lhsT layouts, pre-scaled by S
    ws = (w.transpose(1, 2, 3, 0) * S).astype(np.float32)  # [ci, ky, kx, co]
    # bf16 part: taps ky=0,1 -> [ci, co_t, ci_t, tap, co]
    wbf = (ws[:, 0:2, :, :]
           .reshape(2, 128, 6, 2, 128)        # [ci_t, ci, tap, co_t, co]
           .transpose(1, 3, 0, 2, 4))         # [ci, co_t, ci_t, tap, co]
    wbf = np.ascontiguousarray(wbf).astype(ml_dtypes.bfloat16)
    # fp8 part: taps ky=2 -> [ci, co_t, kx, ci_t, co]
    w8 = (ws[:, 2, :, :]
          .reshape(2, 128, 3, 2, 128)         # [ci_t, ci, kx, co_t, co]
          .transpose(1, 3, 2, 0, 4))          # [ci, co_t, kx, ci_t, co]
    w8 = np.ascontiguousarray(w8).astype(ml_dtypes.float8_e4m3)

    nc = _get_nc()
    bnp = np.ascontiguousarray(
        np.stack([scale, shift], axis=-1).reshape(2, 128, 2).astype(np.float32))
    in_maps = [
        {
            "x": np.ascontiguousarray(x[i * NB:(i + 1) * NB]),
            "wb": wbf,
            "w8": w8,
            "bnp": bnp,
        }
        for i in range(N_CORES)
    ]
    res = run_bass_kernel_spmd(nc, in_maps, core_ids=list(range(N_CORES)),
                               trace=_trace)
    LAST_RESULTS = res
    y = np.concatenate([r["y"] for r in res.results], axis=0)
    return y
